# revision 11
# baseline (speedup 1.0000x reference)
"""Trainium2 Bass kernel for a dense transformer block (pre-LN, MHA + MLP).

Sharding: data-parallel over batch — B=8 batch elements, one per NeuronCore.
Each core runs the full block on its [1024, 768] slice; no collectives.

Precision plan (validated on CPU against the fp32 reference, end-to-end
rel err ~5.8e-3 vs the 2e-2 gate):
  - Attention side entirely fp8e4m3 with power-of-2 scales folded into
    existing eviction ops: qkv, scores, p=exp(logits), p@v, proj.
    qkv / p@v / proj run as DoubleRow matmuls (two 128-deep k-tiles per
    instruction, 1 cycle/row on HW).
  - Softmax shift: exp(logit - 2) globally — cancels in normalization,
    keeps exp outputs under fp8e4's max of 240.
  - MLP in bf16 (fp8 there fails the error budget), fp32 PSUM.
  - Residual stream (x_tm) stays fp32 throughout.

Engine balance (this revision):
  - exp is the attention bottleneck (12.6M logits/core ~ 100us on the
    Activation engine alone). 3/8 of exp tiles run on the DVE instead, as
    a one-instruction Schraudolph exp in the fp8e4m3 bit domain:
    bits = RNE(A*logit + B) with a saturating fp32->uint8 convert
    (negatives clamp to 0 = exact underflow; RNE quantization of the bit
    grid matches direct e4m3 rounding). bitcast uint8 -> fp8e4.
  - U(mcp) is skewed 2 iterations behind scores so exp latency never
    stalls the PE; softmax sums come from the ones-column of v_aug, the
    reciprocal is broadcast across partitions by GpSimd (PE K=1 broadcast
    matmuls removed).
  - normalize (recip+mul) for head (i) is emitted during head (i+1)'s
    score loop so the DVE stream never blocks on U completion.
  - k_pad zero-fill / v_aug ones live on GpSimd; q and v PSUM evictions
    on Scalar, k on DVE; LN transposes evict via one batched copy per
    chunk ([128, 768] from a single PSUM bank) instead of 6 copies.
  - LN1 runs chunks 0-3, then qkv for the first 512-token slice, then
    chunks 4-7 (DVE overlaps PE), then the second slice; proj, LN2 and
    the residual add (GpSimd) interleave per token chunk.

Scales: h1*16, w_attn*32 (host), q*8, k*8, v*16, p*1, attn*16; proj psum
carries 16*32=512x, removed during the residual eviction.
"""

import math
import os
import sys
from contextlib import ExitStack

import numpy as np

for _p in ("/opt/trn_rl_repo",):
    if os.path.isdir(_p) and _p not in sys.path:
        sys.path.insert(0, _p)

import concourse.bass as bass  # noqa: E402
import concourse.mybir as mybir  # noqa: E402
import concourse.tile as tile  # noqa: E402
from concourse import bacc  # noqa: E402
from concourse.masks import make_identity  # noqa: E402

B, SEQ, C, H, HD, HID = 8, 1024, 768, 12, 64, 3072
P = 128
FP = mybir.dt.float32
F8 = mybir.dt.float8e4
U8 = mybir.dt.uint8
BF = mybir.dt.bfloat16
DR = mybir.MatmulPerfMode.DoubleRow
TC_N = SEQ // P          # 8 token chunks of 128
NW = 512                 # wide token slice for matmul free dim
NWN = SEQ // NW          # 2
KC = C // P              # 6 contraction chunks over C
KCP = KC // 2            # 3 contraction chunk-pairs (DoubleRow)
QKF = 2 * C // P         # 12 feature chunks covering q then k
HC_N = HID // P          # 24 hidden chunks
PAIRS = H // 2           # 6 head pairs (2 heads share a 128-partition tile)
CS_W = C // 2            # 384-wide output slices for token-major matmuls
MCP = TC_N // 2          # 4 key-chunk pairs
SCALE = HD ** -0.5
EPS = 1e-6
AF = mybir.ActivationFunctionType
OP = mybir.AluOpType

# fp8 power-of-2 scales
S_H = 16.0               # LN1 output
S_W = 32.0               # attention-side weights (applied on host)
S_QK = 8.0               # q and k
S_V = 16.0               # v (and hence attn out after normalize)
SHIFT = 2.0              # global logit downshift; cancels in softmax

# Schraudolph exp in e4m3 bit domain: bits = RNE(A*logit + B), uint8-sat.
EXP_SCALE = SCALE / (S_QK * S_QK)
A_DVE = 8.0 * math.log2(math.e) * EXP_SCALE
B_DVE = 56.0 - 8.0 * math.log2(math.e) * SHIFT - 0.344

# 80B head slots in v_aug: dual-fp8 ldweights needs even, 16B-aligned
# k-tile strides and even offsets; 65 of the 80 bytes are used (64 v dims
# + ones column). 48B row tail pad lets the U lhsT read a full 128-wide
# slot (garbage output rows 65..127) keeping a 128-partition output.
VSL = 80


def _build(ln_affine: bool, proj_bias: bool):
    nc = bacc.Bacc("TRN2", debug=False)
    x_d = nc.dram_tensor("x", [SEQ, C], FP, kind="ExternalInput").ap()
    qkvw_d = nc.dram_tensor("qkv_w", [C, 3 * C], F8, kind="ExternalInput").ap()
    projw_d = nc.dram_tensor("proj_w", [C, C], F8, kind="ExternalInput").ap()
    fc1w_d = nc.dram_tensor("fc1_w", [C, HID], BF, kind="ExternalInput").ap()
    fc2w_d = nc.dram_tensor("fc2_w", [HID, C], BF, kind="ExternalInput").ap()
    fc1b_d = nc.dram_tensor("fc1_b", [HID], FP, kind="ExternalInput").ap()
    lnp = {}
    if ln_affine:
        for nm in ("ln1_g", "ln1_b", "ln2_g", "ln2_b"):
            lnp[nm] = nc.dram_tensor(nm, [C], FP, kind="ExternalInput").ap()
    if proj_bias:
        lnp["proj_b"] = nc.dram_tensor("proj_b", [C], FP, kind="ExternalInput").ap()
    out_d = nc.dram_tensor("out", [SEQ, C], FP, kind="ExternalOutput").ap()

    with tile.TileContext(nc) as tc:
        with ExitStack() as ctx:
            _body(nc, tc, ctx, x_d, qkvw_d, projw_d, fc1w_d, fc2w_d, fc1b_d,
                  lnp, out_d, ln_affine, proj_bias)
    nc.compile()
    return nc


def _body(nc, tc, ctx, x_d, qkvw_d, projw_d, fc1w_d, fc2w_d, fc1b_d, lnp,
          out_d, ln_affine, proj_bias):
    v, s, te, dma, gp = nc.vector, nc.scalar, nc.tensor, nc.sync, nc.gpsimd

    # ---------- persistent pool ----------
    p0 = ctx.enter_context(tc.tile_pool(name="p0", bufs=1))
    x_tm = p0.tile([P, TC_N, C], FP)       # holds x, then x1, then out
    x_src = x_d.rearrange("(tc p) c -> p tc c", p=P)
    for tcx in range(TC_N):
        dma.dma_start(out=x_tm[:, tcx, :], in_=x_src[:, tcx, :])
    identb = p0.tile([P, P], BF)
    make_identity(nc, identb)
    eps_t = p0.tile([P, 1], FP)
    v.memset(eps_t, EPS)
    ones_col = p0.tile([P, H], FP)
    v.memset(ones_col, 1.0)
    nshift_t = p0.tile([P, 1], FP)
    v.memset(nshift_t, -SHIFT)
    fc1b_t = p0.tile([P, HC_N], FP)
    dma.dma_start(out=fc1b_t, in_=fc1b_d.rearrange("(hc p) -> p hc", p=P))

    def bcast_c(pool, name):
        if name not in lnp:
            return None
        t = pool.tile([P, C], FP, name=name + "_bc", tag=name, bufs=1)
        src = lnp[name]
        ap = bass.AP(tensor=src.tensor, offset=src.offset, ap=[[0, P], src.ap[0]])
        gp.dma_start(out=t, in_=ap)
        return t

    # LN chunk: stats + apply + batched transpose into h_fm[:, :, tcx*P:].
    def ln_chunk(sc, tcx, x_big, h_fm, stat_pool, h_pool, tpsum, g_t, b_t,
                 tp_dt, out_scale):
        xs = x_big[:, tcx, :]
        affine = g_t is not None or b_t is not None
        stats = stat_pool.tile([P, 3, 6], FP, tag="stats", name=f"stats{sc}{tcx}")
        for i in range(3):
            v.bn_stats(out=stats[:, i, :], in_=xs[:, i * 256:(i + 1) * 256])
        mv = stat_pool.tile([P, 2], FP, tag="mv", name=f"mv{sc}{tcx}")
        v.bn_aggr(out=mv, in_=stats)
        rstd = stat_pool.tile([P, 1], FP, tag="rstd", name=f"rstd{sc}{tcx}")
        s.activation(out=rstd, in_=mv[:, 1:2], func=AF.Sqrt, bias=eps_t, scale=1.0)
        v.reciprocal(out=rstd, in_=rstd)
        nb = stat_pool.tile([P, 1], FP, tag="nb", name=f"nb{sc}{tcx}")
        sc_out = 1.0 if affine else out_scale
        v.tensor_scalar(out=nb, in0=mv[:, 0:1], scalar1=rstd, scalar2=-sc_out,
                        op0=OP.mult, op1=OP.mult)
        if sc_out != 1.0:
            rs2 = stat_pool.tile([P, 1], FP, tag="rs2", name=f"rs2{sc}{tcx}")
            v.tensor_scalar_mul(out=rs2, in0=rstd, scalar1=sc_out)
            rstd = rs2
        h_t = h_pool.tile([P, C], FP if affine else tp_dt, tag="h_tm",
                          name=f"htm{sc}{tcx}")
        s.activation(out=h_t, in_=xs, func=AF.Identity, bias=nb, scale=rstd)
        if affine:
            if g_t is not None:
                v.tensor_mul(out=h_t, in0=h_t, in1=g_t)
            if b_t is not None:
                v.tensor_add(out=h_t, in0=h_t, in1=b_t)
            h8 = h_pool.tile([P, C], tp_dt, tag="h_tm8", name=f"htm8{sc}{tcx}")
            v.tensor_scalar_mul(out=h8, in0=h_t, scalar1=out_scale)
            h_t = h8
        if os.environ.get("KNOBT"):
            for fc in range(KC):
                p1 = tpsum.tile([P, P], tp_dt, tag="tp1", name=f"tq{sc}{tcx}{fc}")
                te.transpose(p1, h_t[:, fc * P:(fc + 1) * P], identb)
                v.tensor_copy(out=h_fm[:, fc, tcx * P:(tcx + 1) * P], in_=p1)
        else:
            pst = tpsum.tile([P, KC, P], tp_dt, tag="tp", name=f"tp{sc}{tcx}")
            for fc in range(KC):
                te.transpose(pst[:, fc, :], h_t[:, fc * P:(fc + 1) * P], identb)
            v.tensor_copy(out=h_fm[:, :, tcx * P:(tcx + 1) * P], in_=pst)

    # ---------- stage 1: qkv + attention + proj inputs ----------
    s1p = ctx.enter_context(tc.tile_pool(name="s1", bufs=1))
    q_fm = s1p.tile([P, PAIRS, SEQ], F8)      # q packed 2 heads/tile
    # k zero-padded per head: full-K=128 scores matmuls keep the whole
    # PE array active (HAM otherwise holds the clock at 1.2 GHz).
    k_pad = s1p.tile([P, H, SEQ], F8)
    v_aug = s1p.tile([P, TC_N, H * VSL + 48], F8)  # v + ones col per head
    attn_fm = s1p.tile([P, KC, SEQ], F8)
    pw = s1p.tile([P, KC, C], F8)
    dma.dma_start(out=pw, in_=projw_d.rearrange("(kc p) c -> p kc c", p=P))

    # attention-operand padding on the otherwise idle GpSimd engine
    # (KNOGPINIT=1 falls back to the DVE for these)
    ini = v if os.environ.get("KNOGPINIT") else gp
    for h in range(H):
        lo, hi = (64, 128) if h % 2 == 0 else (0, 64)
        ini.memset(k_pad[lo:hi, h, :], 0.0)
    for tcx in range(TC_N):
        # zero-fill covers the 15B inter-slot + 48B tail garbage regions so
        # the U lhsT window reads defined data (output rows 65+ unused).
        ini.memset(v_aug[:, tcx, :], 0.0)
        va = v_aug[:, tcx, 0:H * VSL].rearrange("p (h e) -> p h e", e=VSL)
        ini.tensor_copy(out=va[:, 0:H, 64:65], in_=ones_col)

    # ----- LN1 -> h_fm (fp8, x16) interleaved with qkv (DoubleRow) -----
    with ExitStack() as sa:
        sap = sa.enter_context(tc.tile_pool(name="sa", bufs=1))
        h_fm = sap.tile([P, KC, SEQ], F8)
        wqk_pool = sa.enter_context(tc.tile_pool(name="wqk", bufs=QKF))
        wv_pool = sa.enter_context(tc.tile_pool(name="wv", bufs=2))
        stat_pool = sa.enter_context(tc.tile_pool(name="st1", bufs=4))
        h_pool = sa.enter_context(tc.tile_pool(name="htm1", bufs=3))
        tpsum = sa.enter_context(tc.tile_pool(name="tp1", bufs=2, space="PSUM"))
        qkps = sa.enter_context(tc.tile_pool(name="qkps", bufs=3, space="PSUM"))
        vps = sa.enter_context(tc.tile_pool(name="vps", bufs=1, space="PSUM"))
        ln1_g, ln1_b = bcast_c(sap, "ln1_g"), bcast_c(sap, "ln1_b")

        qkv_r = qkvw_d.rearrange("(kc p) f -> p kc f", p=P)
        wqks = []
        for f in range(QKF):
            wqk = wqk_pool.tile([P, KC, P], F8, tag="wqk", name=f"wqk{f}")
            dma.dma_start(out=wqk, in_=qkv_r[:, :, f * P:(f + 1) * P])
            wqks.append(wqk)
        wvs = []
        for vs in range(2):
            wv = wv_pool.tile([P, KC, CS_W], F8, tag="wv", name=f"wv{vs}")
            dma.dma_start(
                out=wv, in_=qkv_r[:, :, 2 * C + vs * CS_W:2 * C + (vs + 1) * CS_W])
            wvs.append(wv)

        def v_mm(tcx):
            pss = [vps.tile([P, CS_W], FP, tag=f"vps{vs}", name=f"vp{tcx}{vs}")
                   for vs in range(2)]
            for kcp in range(KCP):
                for vs in range(2):  # consecutive matmuls share lhsT
                    te.matmul(pss[vs],
                              lhsT=(h_fm[:, 2 * kcp:2 * kcp + 2,
                                         tcx * P:(tcx + 1) * P]),
                              rhs=(wvs[vs][:, 2 * kcp:2 * kcp + 2, :]),
                              start=kcp == 0, stop=kcp == KCP - 1,
                              perf_mode=DR)
            dst = v_aug[:, tcx, 0:H * VSL].rearrange("p (h e) -> p h e", e=VSL)
            for vs in range(2):
                # psum = 512*v; evict at *16 => /32 (on Scalar)
                s.activation(out=dst[:, vs * 6:(vs + 1) * 6, 0:64],
                             in_=pss[vs], func=AF.Copy,
                             scale=S_V / (S_W * S_H))

        def qkv_pass(nn):
            nsl = slice(nn * NW, (nn + 1) * NW)
            for f in range(QKF):
                ps = qkps.tile([P, NW], FP, tag="qkps", name=f"qkp{f}{nn}")
                for kcp in range(KCP):
                    te.matmul(ps, lhsT=(wqks[f][:, 2 * kcp:2 * kcp + 2, :]),
                              rhs=(h_fm[:, 2 * kcp:2 * kcp + 2, nsl]),
                              start=kcp == 0, stop=kcp == KCP - 1,
                              perf_mode=DR)
                # psum = (32w)^T (16h) = 512*val; evict at *8 => /64
                if f < PAIRS:
                    s.activation(out=q_fm[:, f, nsl], in_=ps, func=AF.Copy,
                                 scale=S_QK / (S_W * S_H))
                else:
                    pr = f - PAIRS
                    v.tensor_scalar_mul(out=k_pad[0:64, 2 * pr, nsl],
                                        in0=ps[0:64, :],
                                        scalar1=S_QK / (S_W * S_H))
                    v.tensor_scalar_mul(out=k_pad[64:128, 2 * pr + 1, nsl],
                                        in0=ps[64:128, :],
                                        scalar1=S_QK / (S_W * S_H))

        for tcx in range(4):
            ln_chunk(1, tcx, x_tm, h_fm, stat_pool, h_pool, tpsum,
                     ln1_g, ln1_b, BF, S_H)
        for tcx in range(4):
            v_mm(tcx)
        qkv_pass(0)
        for tcx in range(4, TC_N):
            ln_chunk(1, tcx, x_tm, h_fm, stat_pool, h_pool, tpsum,
                     ln1_g, ln1_b, BF, S_H)
        for tcx in range(4, TC_N):
            v_mm(tcx)
        qkv_pass(1)

    # ----- attention: scores -> exp (Scalar/DVE split) -> U (skew 2) -----
    # KEXP: 1 = one-op DVE Schraudolph (fp32->uint8 RNE+saturate on HW;
    # CoreSim models trunc+wrap so sim shows spurious NaNs), 2 = two-op
    # (explicit relu then convert; safe in both), 0 = all exp on Scalar.
    kexp = int(os.environ.get("KEXP", "2"))
    nogpbc = bool(os.environ.get("KNOGPBC"))
    with ExitStack() as sb:
        e_pool = sb.enter_context(tc.tile_pool(name="epool", bufs=4))
        e2_pool = sb.enter_context(tc.tile_pool(name="epool2", bufs=3))
        rt_pool = sb.enter_context(tc.tile_pool(name="rt", bufs=2))
        sps = sb.enter_context(tc.tile_pool(name="sps", bufs=2, space="PSUM"))
        ups = sb.enter_context(
            tc.tile_pool(name="ups", bufs=1 if nogpbc else 2, space="PSUM"))
        if nogpbc:
            rps = sb.enter_context(tc.tile_pool(name="rps", bufs=1, space="PSUM"))
            sums_pool = sb.enter_context(tc.tile_pool(name="sums", bufs=2))
            ones_row = s1p.tile([1, 64], mybir.dt.float32r)
            onef = s1p.tile([1, 64], FP)
            v.memset(onef, 1.0)
            v.tensor_copy(out=ones_row, in_=onef)

        exp_i = 0
        pend = None          # (psU, nsl, pr, hh) awaiting normalize
        rt_cur = None

        def norm_recip(pu, pr_, nn_, hh_):
            rt = rt_pool.tile([64, NW], FP, tag="rtb", name=f"rb_{pr_}{nn_}{hh_}")
            if nogpbc:
                sums = sums_pool.tile([1, NW], mybir.dt.float32r, tag="sums",
                                      name=f"sm_{pr_}{nn_}{hh_}")
                v.tensor_copy(out=sums, in_=pu[64:65, :])
                psr = rps.tile([64, NW], FP, tag="rps", name=f"ps_{pr_}{nn_}{hh_}")
                te.matmul(psr, lhsT=ones_row, rhs=sums, start=True, stop=True)
                v.reciprocal_approx_fast(out=rt, in_=psr)
            else:
                rt1 = rt_pool.tile([1, NW], FP, tag="rt1",
                                   name=f"r1_{pr_}{nn_}{hh_}")
                v.reciprocal_approx_fast(out=rt1, in_=pu[64:65, :])
                gp.partition_broadcast(rt, rt1, channels=64)
            return rt

        for nn in range(NWN):
            nsl = slice(nn * NW, (nn + 1) * NW)
            for pr in range(PAIRS):
                for hh in (0, 1):
                    ha = 2 * pr + hh
                    if pend is not None:
                        rt_cur = norm_recip(pend[0], pend[2], nn, pend[3])
                    psU = ups.tile([P, NW], FP, tag=f"u{hh}",
                                   name=f"u{pr}{nn}{hh}")
                    ets = {}
                    # 2-iteration skew: U(mcp-2) issues after scores(mcp),
                    # giving exp a full ~1us window off the PE critical path.
                    for mcp in range(MCP + 2):
                        if mcp < MCP:
                            scr = sps.tile([P, 2, NW], FP, tag="sps",
                                           name=f"sc{pr}{nn}{hh}{mcp}")
                            for sub in (0, 1):
                                mc = 2 * mcp + sub
                                te.matmul(
                                    scr[:, sub, :],
                                    lhsT=(k_pad[:, ha, mc * P:(mc + 1) * P]),
                                    rhs=(q_fm[:, pr, nsl]),
                                    start=True, stop=True)
                            et = e_pool.tile([P, 2, NW], F8, tag="E",
                                             name=f"E{pr}{nn}{hh}{mcp}")
                            if kexp and (exp_i * 3) % 8 < 3:
                                if kexp == 1:
                                    v.tensor_scalar(out=et.bitcast(U8),
                                                    in0=scr,
                                                    scalar1=A_DVE,
                                                    scalar2=B_DVE,
                                                    op0=OP.mult, op1=OP.add)
                                else:
                                    tb = e2_pool.tile([P, 2, NW], BF, tag="E2",
                                                      name=f"F{pr}{nn}{hh}{mcp}")
                                    v.tensor_scalar(out=tb, in0=scr,
                                                    scalar1=A_DVE,
                                                    scalar2=B_DVE,
                                                    op0=OP.mult, op1=OP.add)
                                    v.tensor_scalar_max(out=et.bitcast(U8),
                                                        in0=tb, scalar1=0.0)
                            else:
                                s.activation(out=et, in_=scr, func=AF.Exp,
                                             scale=EXP_SCALE, bias=nshift_t)
                            exp_i += 1
                            ets[mcp] = et
                        if mcp >= 2:
                            te.matmul(
                                psU,
                                lhsT=(v_aug[:, 2 * (mcp - 2):2 * (mcp - 1),
                                            ha * VSL:ha * VSL + P]),
                                rhs=(ets.pop(mcp - 2)),
                                start=mcp == 2, stop=mcp == MCP + 1,
                                perf_mode=DR)
                    if pend is not None:
                        pu, onsl, opr, ohh = pend
                        v.tensor_mul(out=attn_fm[ohh * 64:(ohh + 1) * 64,
                                                 opr, onsl],
                                     in0=pu[0:64, :], in1=rt_cur)
                    pend = (psU, nsl, pr, hh)
        pu, onsl, opr, ohh = pend
        rt_cur = norm_recip(pu, opr, 9, ohh)
        v.tensor_mul(out=attn_fm[ohh * 64:(ohh + 1) * 64, opr, onsl],
                     in0=pu[0:64, :], in1=rt_cur)

    # ---------- stage 2: proj + LN2 (interleaved per chunk) + MLP ----------
    with ExitStack() as s2:
        s2p = s2.enter_context(tc.tile_pool(name="s2", bufs=1))
        g_fm = s2p.tile([P, HC_N, SEQ], BF)
        w2_pool = s2.enter_context(tc.tile_pool(name="w2", bufs=12))

        with ExitStack() as sc_:
            scp = sc_.enter_context(tc.tile_pool(name="sc", bufs=1))
            h2_fm = scp.tile([P, KC, SEQ], BF)
            w1_pool = sc_.enter_context(tc.tile_pool(name="w1", bufs=3))
            stat2 = sc_.enter_context(tc.tile_pool(name="st2", bufs=4))
            h2_pool = sc_.enter_context(tc.tile_pool(name="htm2", bufs=2))
            ln2_g, ln2_b = bcast_c(scp, "ln2_g"), bcast_c(scp, "ln2_b")

            with ExitStack() as sb2:
                tpsum2 = sb2.enter_context(
                    tc.tile_pool(name="tp2", bufs=2, space="PSUM"))
                pps = sb2.enter_context(
                    tc.tile_pool(name="pps", bufs=3, space="PSUM"))
                ptmp_pool = sb2.enter_context(tc.tile_pool(name="ptmp", bufs=3))
                pbp = sb2.enter_context(tc.tile_pool(name="pbp", bufs=1))
                projb_t = bcast_c(pbp, "proj_b") if proj_bias else None
                pscale = 1.0 / (S_V * S_W)
                for tcx in range(TC_N):
                    pss = [pps.tile([P, CS_W], FP, tag=f"pps{cs}",
                                    name=f"pp{tcx}{cs}") for cs in range(2)]
                    for kcp in range(KCP):
                        for cs in range(2):  # consecutive matmuls share lhsT
                            te.matmul(pss[cs],
                                      lhsT=(attn_fm[:, 2 * kcp:2 * kcp + 2,
                                                    tcx * P:(tcx + 1) * P]),
                                      rhs=(pw[:, 2 * kcp:2 * kcp + 2,
                                              cs * CS_W:(cs + 1) * CS_W]),
                                      start=kcp == 0, stop=kcp == KCP - 1,
                                      perf_mode=DR)
                    for cs in range(2):
                        ps = pss[cs]
                        xsl = x_tm[:, tcx, cs * CS_W:(cs + 1) * CS_W]
                        ptmp = ptmp_pool.tile([P, CS_W], BF, tag=f"pt{cs}",
                                              name=f"pt{tcx}{cs}")
                        s.activation(out=ptmp, in_=ps, func=AF.Copy,
                                     scale=pscale)
                        if projb_t is not None:
                            v.tensor_add(out=ptmp, in0=ptmp,
                                         in1=projb_t[:, cs * CS_W:(cs + 1) * CS_W])
                        addeng = v if os.environ.get("KNOGPADD") else gp
                        addeng.tensor_add(out=xsl, in0=ptmp, in1=xsl)
                    ln_chunk(2, tcx, x_tm, h2_fm, stat2, h2_pool, tpsum2,
                             ln2_g, ln2_b, BF, 1.0)

            fc1_r = fc1w_d.rearrange("(kc p) f -> p kc f", p=P)
            f1ps = sc_.enter_context(tc.tile_pool(name="f1ps", bufs=3, space="PSUM"))
            for hc in range(HC_N):
                w1 = w1_pool.tile([P, KC, P], BF, tag="w1", name=f"w1_{hc}")
                dma.dma_start(out=w1, in_=fc1_r[:, :, hc * P:(hc + 1) * P])
                pss = [f1ps.tile([P, NW], FP, tag=f"f1ps{nn}", name=f"f1p{hc}{nn}")
                       for nn in range(NWN)]
                for kc in range(KC):
                    for nn in range(NWN):  # consecutive matmuls share lhsT
                        te.matmul(pss[nn], lhsT=(w1[:, kc, :]),
                                  rhs=(h2_fm[:, kc, nn * NW:(nn + 1) * NW]),
                                  start=kc == 0, stop=kc == KC - 1)
                for nn in range(NWN):
                    s.activation(out=g_fm[:, hc, nn * NW:(nn + 1) * NW], in_=pss[nn],
                                 func=AF.Gelu, bias=fc1b_t[:, hc:hc + 1], scale=1.0)

        # fc2 in groups of 6 hidden chunks, accumulate into x_tm
        GRP = 6
        fc2_r = fc2w_d.rearrange("(hc p) c -> p hc c", p=P)
        out_r = out_d.rearrange("(tc p) c -> p tc c", p=P)
        with ExitStack() as sd:
            f2ps = sd.enter_context(tc.tile_pool(name="f2ps", bufs=3, space="PSUM"))
            for grp in range(HC_N // GRP):
                hcs = list(range(grp * GRP, (grp + 1) * GRP))
                w2t = {}
                for hc in hcs:
                    w2t[hc] = w2_pool.tile([P, C], BF, tag="w2", name=f"w2_{hc}")
                    dma.dma_start(out=w2t[hc], in_=fc2_r[:, hc, :])
                for tcx in range(TC_N):
                    pss = [f2ps.tile([P, CS_W], FP, tag=f"f2ps{cs}",
                                     name=f"f2p{grp}{tcx}{cs}") for cs in range(2)]
                    for i, hc in enumerate(hcs):
                        for cs in range(2):  # consecutive matmuls share lhsT
                            te.matmul(
                                pss[cs], lhsT=(g_fm[:, hc, tcx * P:(tcx + 1) * P]),
                                rhs=(w2t[hc][:, cs * CS_W:(cs + 1) * CS_W]),
                                start=i == 0, stop=i == GRP - 1)
                    for cs in range(2):
                        xsl = x_tm[:, tcx, cs * CS_W:(cs + 1) * CS_W]
                        v.tensor_add(out=xsl, in0=pss[cs], in1=xsl)
                    if grp == HC_N // GRP - 1:
                        dma.dma_start(out=out_r[:, tcx, :], in_=x_tm[:, tcx, :])


_CACHE = {}
last_results = None


def _get_nc(ln_affine, proj_bias):
    key = (ln_affine, proj_bias)
    if key not in _CACHE:
        _CACHE[key] = _build(*key)
    return _CACHE[key]


def kernel(x, qkv_w, proj_w, proj_b, ln1_g, ln1_b, ln2_g, ln2_b,
           fc1_w, fc1_b, fc2_w, fc2_b):
    global last_results
    import ml_dtypes
    from concourse.bass_utils import run_bass_kernel_spmd

    F8NP = ml_dtypes.float8_e4m3
    BFNP = ml_dtypes.bfloat16
    f32 = lambda a: np.ascontiguousarray(np.asarray(a), dtype=np.float32)
    x = f32(x)
    proj_b, fc1_b, fc2_b = map(f32, (proj_b, fc1_b, fc2_b))
    ln1_g, ln1_b, ln2_g, ln2_b = map(f32, (ln1_g, ln1_b, ln2_g, ln2_b))
    qkv_w8 = np.ascontiguousarray(
        (np.asarray(qkv_w, np.float32) * S_W).astype(F8NP))
    proj_w8 = np.ascontiguousarray(
        (np.asarray(proj_w, np.float32) * S_W).astype(F8NP))
    fc1_wb = np.ascontiguousarray(np.asarray(fc1_w, np.float32).astype(BFNP))
    fc2_wb = np.ascontiguousarray(np.asarray(fc2_w, np.float32).astype(BFNP))

    ln_affine = not (np.all(ln1_g == 1) and np.all(ln1_b == 0)
                     and np.all(ln2_g == 1) and np.all(ln2_b == 0))
    proj_bias = bool(np.any(proj_b != 0))
    nc = _get_nc(ln_affine, proj_bias)

    common = {"qkv_w": qkv_w8, "proj_w": proj_w8, "fc1_w": fc1_wb,
              "fc2_w": fc2_wb, "fc1_b": fc1_b}
    if ln_affine:
        common.update({"ln1_g": ln1_g, "ln1_b": ln1_b,
                       "ln2_g": ln2_g, "ln2_b": ln2_b})
    if proj_bias:
        common["proj_b"] = proj_b
    in_maps = [dict(common, x=np.ascontiguousarray(x[b])) for b in range(B)]

    res = run_bass_kernel_spmd(nc, in_maps, core_ids=list(range(B)))
    last_results = res
    out = np.stack([r["out"] for r in res.results], axis=0)
    # fc2_b commutes past the final residual add — fold on host.
    return (out + fc2_b[None, None, :]).astype(np.float32)


# revision 17
# speedup vs baseline: 1.0455x; 1.0455x over previous
"""Trainium2 Bass kernel for a dense transformer block (pre-LN, MHA + MLP).

Sharding: data-parallel over batch — B=8 batch elements, one per NeuronCore.
Each core runs the full block on its [1024, 768] slice; no collectives.

Precision plan (validated on CPU against the fp32 reference, end-to-end
rel err ~5.8e-3 vs the 2e-2 gate):
  - Attention side entirely fp8e4m3 with power-of-2 scales folded into
    existing eviction ops: qkv, scores, p=exp(logits), p@v, proj.
    qkv / p@v / proj run as DoubleRow matmuls (two 128-deep k-tiles per
    instruction, 1 cycle/row on HW).
  - Softmax shift: exp(logit - 2) globally — cancels in normalization,
    keeps exp outputs under fp8e4's max of 240.
  - MLP in bf16 (fp8 there fails the error budget), fp32 PSUM.
  - Residual stream (x_tm) stays fp32 throughout.

Engine balance (this revision):
  - exp is the attention bottleneck (12.6M logits/core ~ 100us on the
    Activation engine alone). 3/8 of exp tiles run on the DVE instead, as
    a one-instruction Schraudolph exp in the fp8e4m3 bit domain:
    bits = RNE(A*logit + B) with a saturating fp32->uint8 convert
    (negatives clamp to 0 = exact underflow; RNE quantization of the bit
    grid matches direct e4m3 rounding). bitcast uint8 -> fp8e4.
  - U(mcp) is skewed 2 iterations behind scores so exp latency never
    stalls the PE; softmax sums come from the ones-column of v_aug, the
    reciprocal is broadcast across partitions by GpSimd (PE K=1 broadcast
    matmuls removed).
  - normalize (recip+mul) for head (i) is emitted during head (i+1)'s
    score loop so the DVE stream never blocks on U completion.
  - k_pad zero-fill / v_aug ones live on GpSimd; q and v PSUM evictions
    on Scalar, k on DVE; LN transposes evict via one batched copy per
    chunk ([128, 768] from a single PSUM bank) instead of 6 copies.
  - LN1 runs chunks 0-3, then qkv for the first 512-token slice, then
    chunks 4-7 (DVE overlaps PE), then the second slice; proj, LN2 and
    the residual add (GpSimd) interleave per token chunk.

Scales: h1*16, w_attn*32 (host), q*8, k*8, v*16, p*1, attn*16; proj psum
carries 16*32=512x, removed during the residual eviction.
"""

import math
import os
import sys
from contextlib import ExitStack

import numpy as np

for _p in ("/opt/trn_rl_repo",):
    if os.path.isdir(_p) and _p not in sys.path:
        sys.path.insert(0, _p)

import concourse.bass as bass  # noqa: E402
import concourse.mybir as mybir  # noqa: E402
import concourse.tile as tile  # noqa: E402
from concourse import bacc  # noqa: E402
from concourse.masks import make_identity  # noqa: E402

B, SEQ, C, H, HD, HID = 8, 1024, 768, 12, 64, 3072
P = 128
FP = mybir.dt.float32
F8 = mybir.dt.float8e4
U8 = mybir.dt.uint8
BF = mybir.dt.bfloat16
DR = mybir.MatmulPerfMode.DoubleRow
TC_N = SEQ // P          # 8 token chunks of 128
NW = 512                 # wide token slice for matmul free dim
NWN = SEQ // NW          # 2
KC = C // P              # 6 contraction chunks over C
KCP = KC // 2            # 3 contraction chunk-pairs (DoubleRow)
QKF = 2 * C // P         # 12 feature chunks covering q then k
HC_N = HID // P          # 24 hidden chunks
PAIRS = H // 2           # 6 head pairs (2 heads share a 128-partition tile)
CS_W = C // 2            # 384-wide output slices for token-major matmuls
MCP = TC_N // 2          # 4 key-chunk pairs
SCALE = HD ** -0.5
EPS = 1e-6
AF = mybir.ActivationFunctionType
OP = mybir.AluOpType

# fp8 power-of-2 scales
S_H = 16.0               # LN1 output
S_W = 32.0               # attention-side weights (applied on host)
S_QK = 8.0               # q and k
S_V = 16.0               # v (and hence attn out after normalize)
SHIFT = 2.0              # global logit downshift; cancels in softmax

# Schraudolph exp in e4m3 bit domain: bits = RNE(A*logit + B), uint8-sat.
EXP_SCALE = SCALE / (S_QK * S_QK)
A_DVE = 8.0 * math.log2(math.e) * EXP_SCALE
B_DVE = 56.0 - 8.0 * math.log2(math.e) * SHIFT - 0.344

# 80B head slots in v_aug: dual-fp8 ldweights needs even, 16B-aligned
# k-tile strides and even offsets; 65 of the 80 bytes are used (64 v dims
# + ones column). 48B row tail pad lets the U lhsT read a full 128-wide
# slot (garbage output rows 65..127) keeping a 128-partition output.
VSL = 80


def _build(ln_affine: bool, proj_bias: bool):
    nc = bacc.Bacc("TRN2", debug=False)
    x_d = nc.dram_tensor("x", [SEQ, C], FP, kind="ExternalInput").ap()
    qkvw_d = nc.dram_tensor("qkv_w", [C, 3 * C], F8, kind="ExternalInput").ap()
    projw_d = nc.dram_tensor("proj_w", [C, C], F8, kind="ExternalInput").ap()
    fc1w_d = nc.dram_tensor("fc1_w", [C, HID], BF, kind="ExternalInput").ap()
    fc2w_d = nc.dram_tensor("fc2_w", [HID, C], BF, kind="ExternalInput").ap()
    fc1b_d = nc.dram_tensor("fc1_b", [HID], FP, kind="ExternalInput").ap()
    lnp = {}
    if ln_affine:
        for nm in ("ln1_g", "ln1_b", "ln2_g", "ln2_b"):
            lnp[nm] = nc.dram_tensor(nm, [C], FP, kind="ExternalInput").ap()
    if proj_bias:
        lnp["proj_b"] = nc.dram_tensor("proj_b", [C], FP, kind="ExternalInput").ap()
    out_d = nc.dram_tensor("out", [SEQ, C], FP, kind="ExternalOutput").ap()

    with tile.TileContext(nc) as tc:
        with ExitStack() as ctx:
            _body(nc, tc, ctx, x_d, qkvw_d, projw_d, fc1w_d, fc2w_d, fc1b_d,
                  lnp, out_d, ln_affine, proj_bias)
    nc.compile()
    return nc


def _body(nc, tc, ctx, x_d, qkvw_d, projw_d, fc1w_d, fc2w_d, fc1b_d, lnp,
          out_d, ln_affine, proj_bias):
    v, s, te, dma, gp = nc.vector, nc.scalar, nc.tensor, nc.sync, nc.gpsimd

    # ---------- persistent pool ----------
    p0 = ctx.enter_context(tc.tile_pool(name="p0", bufs=1))
    x_tm = p0.tile([P, TC_N, C], FP)       # holds x, then x1, then out
    x_src = x_d.rearrange("(tc p) c -> p tc c", p=P)
    for tcx in range(TC_N):
        dma.dma_start(out=x_tm[:, tcx, :], in_=x_src[:, tcx, :])
    identb = p0.tile([P, P], BF)
    make_identity(nc, identb)
    eps_t = p0.tile([P, 1], FP)
    v.memset(eps_t, EPS)
    ones_col = p0.tile([P, H], FP)
    v.memset(ones_col, 1.0)
    nshift_t = p0.tile([P, 1], FP)
    v.memset(nshift_t, -SHIFT)
    fc1b_t = p0.tile([P, HC_N], FP)
    dma.dma_start(out=fc1b_t, in_=fc1b_d.rearrange("(hc p) -> p hc", p=P))

    def bcast_c(pool, name):
        if name not in lnp:
            return None
        t = pool.tile([P, C], FP, name=name + "_bc", tag=name, bufs=1)
        src = lnp[name]
        ap = bass.AP(tensor=src.tensor, offset=src.offset, ap=[[0, P], src.ap[0]])
        gp.dma_start(out=t, in_=ap)
        return t

    # LN chunk: stats + apply + batched transpose into h_fm[:, :, tcx*P:].
    def ln_chunk(sc, tcx, x_big, h_fm, stat_pool, h_pool, tpsum, g_t, b_t,
                 tp_dt, out_scale):
        xs = x_big[:, tcx, :]
        affine = g_t is not None or b_t is not None
        stats = stat_pool.tile([P, 3, 6], FP, tag="stats", name=f"stats{sc}{tcx}")
        for i in range(3):
            v.bn_stats(out=stats[:, i, :], in_=xs[:, i * 256:(i + 1) * 256])
        mv = stat_pool.tile([P, 2], FP, tag="mv", name=f"mv{sc}{tcx}")
        v.bn_aggr(out=mv, in_=stats)
        rstd = stat_pool.tile([P, 1], FP, tag="rstd", name=f"rstd{sc}{tcx}")
        s.activation(out=rstd, in_=mv[:, 1:2], func=AF.Sqrt, bias=eps_t, scale=1.0)
        v.reciprocal(out=rstd, in_=rstd)
        nb = stat_pool.tile([P, 1], FP, tag="nb", name=f"nb{sc}{tcx}")
        sc_out = 1.0 if affine else out_scale
        v.tensor_scalar(out=nb, in0=mv[:, 0:1], scalar1=rstd, scalar2=-sc_out,
                        op0=OP.mult, op1=OP.mult)
        if sc_out != 1.0:
            rs2 = stat_pool.tile([P, 1], FP, tag="rs2", name=f"rs2{sc}{tcx}")
            v.tensor_scalar_mul(out=rs2, in0=rstd, scalar1=sc_out)
            rstd = rs2
        h_t = h_pool.tile([P, C], FP if affine else tp_dt, tag="h_tm",
                          name=f"htm{sc}{tcx}")
        s.activation(out=h_t, in_=xs, func=AF.Identity, bias=nb, scale=rstd)
        if affine:
            if g_t is not None:
                v.tensor_mul(out=h_t, in0=h_t, in1=g_t)
            if b_t is not None:
                v.tensor_add(out=h_t, in0=h_t, in1=b_t)
            h8 = h_pool.tile([P, C], tp_dt, tag="h_tm8", name=f"htm8{sc}{tcx}")
            v.tensor_scalar_mul(out=h8, in0=h_t, scalar1=out_scale)
            h_t = h8
        if os.environ.get("KNOBT"):
            for fc in range(KC):
                p1 = tpsum.tile([P, P], tp_dt, tag="tp1", name=f"tq{sc}{tcx}{fc}")
                te.transpose(p1, h_t[:, fc * P:(fc + 1) * P], identb)
                v.tensor_copy(out=h_fm[:, fc, tcx * P:(tcx + 1) * P], in_=p1)
        else:
            pst = tpsum.tile([P, KC, P], tp_dt, tag="tp", name=f"tp{sc}{tcx}")
            for fc in range(KC):
                te.transpose(pst[:, fc, :], h_t[:, fc * P:(fc + 1) * P], identb)
            v.tensor_copy(out=h_fm[:, :, tcx * P:(tcx + 1) * P], in_=pst)

    # ---------- stage 1: qkv + attention + proj inputs ----------
    s1p = ctx.enter_context(tc.tile_pool(name="s1", bufs=1))
    q_fm = s1p.tile([P, PAIRS, SEQ], F8)      # q packed 2 heads/tile
    # k zero-padded per head: full-K=128 scores matmuls keep the whole
    # PE array active (HAM otherwise holds the clock at 1.2 GHz).
    k_pad = s1p.tile([P, H, SEQ], F8)
    v_aug = s1p.tile([P, TC_N, H * VSL + 48], F8)  # v + ones col per head
    attn_fm = s1p.tile([P, KC, SEQ], F8)
    pw = s1p.tile([P, KC, C], F8)
    dma.dma_start(out=pw, in_=projw_d.rearrange("(kc p) c -> p kc c", p=P))

    # attention-operand padding on the otherwise idle GpSimd engine
    # (KNOGPINIT=1 falls back to the DVE for these)
    ini = v if os.environ.get("KNOGPINIT") else gp
    for h in range(H):
        lo, hi = (64, 128) if h % 2 == 0 else (0, 64)
        ini.memset(k_pad[lo:hi, h, :], 0.0)
    for tcx in range(TC_N):
        # zero-fill covers the 15B inter-slot + 48B tail garbage regions so
        # the U lhsT window reads defined data (output rows 65+ unused).
        ini.memset(v_aug[:, tcx, :], 0.0)
        va = v_aug[:, tcx, 0:H * VSL].rearrange("p (h e) -> p h e", e=VSL)
        ini.tensor_copy(out=va[:, 0:H, 64:65], in_=ones_col)

    # ----- LN1 -> h_fm (fp8, x16) interleaved with qkv (DoubleRow) -----
    with ExitStack() as sa:
        sap = sa.enter_context(tc.tile_pool(name="sa", bufs=1))
        h_fm = sap.tile([P, KC, SEQ], F8)
        wqk_pool = sa.enter_context(tc.tile_pool(name="wqk", bufs=QKF))
        wv_pool = sa.enter_context(tc.tile_pool(name="wv", bufs=2))
        stat_pool = sa.enter_context(tc.tile_pool(name="st1", bufs=4))
        h_pool = sa.enter_context(tc.tile_pool(name="htm1", bufs=3))
        tpsum = sa.enter_context(tc.tile_pool(name="tp1", bufs=2, space="PSUM"))
        qkps = sa.enter_context(tc.tile_pool(name="qkps", bufs=2, space="PSUM"))
        vps = sa.enter_context(tc.tile_pool(name="vps", bufs=2, space="PSUM"))
        ln1_g, ln1_b = bcast_c(sap, "ln1_g"), bcast_c(sap, "ln1_b")

        qkv_r = qkvw_d.rearrange("(kc p) f -> p kc f", p=P)
        wqks = []
        for f in range(QKF):
            wqk = wqk_pool.tile([P, KC, P], F8, tag="wqk", name=f"wqk{f}")
            dma.dma_start(out=wqk, in_=qkv_r[:, :, f * P:(f + 1) * P])
            wqks.append(wqk)
        wvs = []
        for vs in range(2):
            wv = wv_pool.tile([P, KC, CS_W], F8, tag="wv", name=f"wv{vs}")
            dma.dma_start(
                out=wv, in_=qkv_r[:, :, 2 * C + vs * CS_W:2 * C + (vs + 1) * CS_W])
            wvs.append(wv)

        def v_mm(tcx):
            pss = [vps.tile([P, CS_W], FP, tag=f"vps{vs}", name=f"vp{tcx}{vs}")
                   for vs in range(2)]
            for kcp in range(KCP):
                for vs in range(2):  # consecutive matmuls share lhsT
                    te.matmul(pss[vs],
                              lhsT=(h_fm[:, 2 * kcp:2 * kcp + 2,
                                         tcx * P:(tcx + 1) * P]),
                              rhs=(wvs[vs][:, 2 * kcp:2 * kcp + 2, :]),
                              start=kcp == 0, stop=kcp == KCP - 1,
                              perf_mode=DR)
            dst = v_aug[:, tcx, 0:H * VSL].rearrange("p (h e) -> p h e", e=VSL)
            for vs in range(2):
                # psum = 512*v; evict at *16 => /32 (on Scalar)
                s.activation(out=dst[:, vs * 6:(vs + 1) * 6, 0:64],
                             in_=pss[vs], func=AF.Copy,
                             scale=S_V / (S_W * S_H))

        def qkv_pass(nn):
            nsl = slice(nn * NW, (nn + 1) * NW)
            for f in range(QKF):
                ps = qkps.tile([P, NW], FP, tag="qkps", name=f"qkp{f}{nn}")
                for kcp in range(KCP):
                    te.matmul(ps, lhsT=(wqks[f][:, 2 * kcp:2 * kcp + 2, :]),
                              rhs=(h_fm[:, 2 * kcp:2 * kcp + 2, nsl]),
                              start=kcp == 0, stop=kcp == KCP - 1,
                              perf_mode=DR)
                # psum = (32w)^T (16h) = 512*val; evict at *8 => /64
                if f < PAIRS:
                    s.activation(out=q_fm[:, f, nsl], in_=ps, func=AF.Copy,
                                 scale=S_QK / (S_W * S_H))
                else:
                    pr = f - PAIRS
                    v.tensor_scalar_mul(out=k_pad[0:64, 2 * pr, nsl],
                                        in0=ps[0:64, :],
                                        scalar1=S_QK / (S_W * S_H))
                    v.tensor_scalar_mul(out=k_pad[64:128, 2 * pr + 1, nsl],
                                        in0=ps[64:128, :],
                                        scalar1=S_QK / (S_W * S_H))

        for tcx in range(4):
            ln_chunk(1, tcx, x_tm, h_fm, stat_pool, h_pool, tpsum,
                     ln1_g, ln1_b, BF, S_H)
        for tcx in range(4):
            v_mm(tcx)
        qkv_pass(0)
        for tcx in range(4, TC_N):
            ln_chunk(1, tcx, x_tm, h_fm, stat_pool, h_pool, tpsum,
                     ln1_g, ln1_b, BF, S_H)
        for tcx in range(4, TC_N):
            v_mm(tcx)
        qkv_pass(1)

    # ----- attention: scores -> exp (Scalar/DVE split) -> U (skew 2) -----
    # KEXP: 1 = one-op DVE Schraudolph (fp32->uint8 RNE+saturate on HW;
    # CoreSim models trunc+wrap so sim shows spurious NaNs), 2 = two-op
    # (explicit relu then convert; safe in both), 0 = all exp on Scalar.
    kexp = int(os.environ.get("KEXP", "1"))
    nogpbc = not os.environ.get("KGPBC")
    with ExitStack() as sb:
        e_pool = sb.enter_context(tc.tile_pool(name="epool", bufs=4))
        e2_pool = sb.enter_context(tc.tile_pool(name="epool2", bufs=3))
        rt_pool = sb.enter_context(tc.tile_pool(name="rt", bufs=2))
        sps = sb.enter_context(tc.tile_pool(name="sps", bufs=2, space="PSUM"))
        ups = sb.enter_context(
            tc.tile_pool(name="ups", bufs=1 if nogpbc else 2, space="PSUM"))
        if nogpbc:
            rps = sb.enter_context(tc.tile_pool(name="rps", bufs=2, space="PSUM"))
            sums_pool = sb.enter_context(tc.tile_pool(name="sums", bufs=2))
            ones_row = s1p.tile([1, 64], mybir.dt.float32r)
            onef = s1p.tile([1, 64], FP)
            v.memset(onef, 1.0)
            v.tensor_copy(out=ones_row, in_=onef)

        exp_i = 0
        pend = None          # (psU, nsl, pr, hh) awaiting normalize
        rt_cur = None

        def norm_recip(pu, pr_, nn_, hh_):
            rt = rt_pool.tile([64, NW], FP, tag="rtb", name=f"rb_{pr_}{nn_}{hh_}")
            if nogpbc:
                sums = sums_pool.tile([1, NW], mybir.dt.float32r, tag="sums",
                                      name=f"sm_{pr_}{nn_}{hh_}")
                # sums eviction on Scalar — frees DVE for its exp share
                s.activation(out=sums, in_=pu[64:65, :], func=AF.Copy, scale=1.0)
                psr = rps.tile([64, NW], FP, tag="rps", name=f"ps_{pr_}{nn_}{hh_}")
                te.matmul(psr, lhsT=ones_row, rhs=sums, start=True, stop=True)
                v.reciprocal_approx_fast(out=rt, in_=psr)
            else:
                rt1 = rt_pool.tile([1, NW], FP, tag="rt1",
                                   name=f"r1_{pr_}{nn_}{hh_}")
                v.reciprocal_approx_fast(out=rt1, in_=pu[64:65, :])
                gp.partition_broadcast(rt, rt1, channels=64)
            return rt

        for nn in range(NWN):
            nsl = slice(nn * NW, (nn + 1) * NW)
            for pr in range(PAIRS):
                for hh in (0, 1):
                    ha = 2 * pr + hh
                    if pend is not None:
                        rt_cur = norm_recip(pend[0], pend[2], nn, pend[3])
                    psU = ups.tile([P, NW], FP, tag=f"u{hh}",
                                   name=f"u{pr}{nn}{hh}")
                    ets = {}
                    # 2-iteration skew: U(mcp-2) issues after scores(mcp),
                    # giving exp a full ~1us window off the PE critical path.
                    for mcp in range(MCP + 2):
                        if mcp < MCP:
                            scr = sps.tile([P, 2, NW], FP, tag="sps",
                                           name=f"sc{pr}{nn}{hh}{mcp}")
                            for sub in (0, 1):
                                mc = 2 * mcp + sub
                                te.matmul(
                                    scr[:, sub, :],
                                    lhsT=(k_pad[:, ha, mc * P:(mc + 1) * P]),
                                    rhs=(q_fm[:, pr, nsl]),
                                    start=True, stop=True)
                            et = e_pool.tile([P, 2, NW], F8, tag="E",
                                             name=f"E{pr}{nn}{hh}{mcp}")
                            if kexp and (exp_i * 2) % 5 < 2:
                                if kexp == 1:
                                    v.tensor_scalar(out=et.bitcast(U8),
                                                    in0=scr,
                                                    scalar1=A_DVE,
                                                    scalar2=B_DVE,
                                                    op0=OP.mult, op1=OP.add)
                                else:
                                    tb = e2_pool.tile([P, 2, NW], BF, tag="E2",
                                                      name=f"F{pr}{nn}{hh}{mcp}")
                                    v.tensor_scalar(out=tb, in0=scr,
                                                    scalar1=A_DVE,
                                                    scalar2=B_DVE,
                                                    op0=OP.mult, op1=OP.add)
                                    v.tensor_scalar_max(out=et.bitcast(U8),
                                                        in0=tb, scalar1=0.0)
                            else:
                                s.activation(out=et, in_=scr, func=AF.Exp,
                                             scale=EXP_SCALE, bias=nshift_t)
                            exp_i += 1
                            ets[mcp] = et
                        if mcp >= 2:
                            te.matmul(
                                psU,
                                lhsT=(v_aug[:, 2 * (mcp - 2):2 * (mcp - 1),
                                            ha * VSL:ha * VSL + P]),
                                rhs=(ets.pop(mcp - 2)),
                                start=mcp == 2, stop=mcp == MCP + 1,
                                perf_mode=DR)
                    if pend is not None:
                        pu, onsl, opr, ohh = pend
                        v.tensor_mul(out=attn_fm[ohh * 64:(ohh + 1) * 64,
                                                 opr, onsl],
                                     in0=pu[0:64, :], in1=rt_cur)
                    pend = (psU, nsl, pr, hh)
        pu, onsl, opr, ohh = pend
        rt_cur = norm_recip(pu, opr, 9, ohh)
        v.tensor_mul(out=attn_fm[ohh * 64:(ohh + 1) * 64, opr, onsl],
                     in0=pu[0:64, :], in1=rt_cur)

    # ---------- stage 2: proj + LN2 (interleaved per chunk) + MLP ----------
    with ExitStack() as s2:
        s2p = s2.enter_context(tc.tile_pool(name="s2", bufs=1))
        g_fm = s2p.tile([P, HC_N, SEQ], BF)
        w2_pool = s2.enter_context(tc.tile_pool(name="w2", bufs=12))

        with ExitStack() as sc_:
            scp = sc_.enter_context(tc.tile_pool(name="sc", bufs=1))
            h2_fm = scp.tile([P, KC, SEQ], BF)
            w1_pool = sc_.enter_context(tc.tile_pool(name="w1", bufs=3))
            stat2 = sc_.enter_context(tc.tile_pool(name="st2", bufs=4))
            h2_pool = sc_.enter_context(tc.tile_pool(name="htm2", bufs=2))
            ln2_g, ln2_b = bcast_c(scp, "ln2_g"), bcast_c(scp, "ln2_b")

            fc1_r = fc1w_d.rearrange("(kc p) f -> p kc f", p=P)
            f1ps = sc_.enter_context(
                tc.tile_pool(name="f1ps", bufs=2, space="PSUM"))

            def fc1_chunk(hc, nn, pass_):
                w1 = w1_pool.tile([P, KC, P], BF, tag="w1",
                                  name=f"w1_{hc}_{pass_}")
                dma.dma_start(out=w1, in_=fc1_r[:, :, hc * P:(hc + 1) * P])
                ps = f1ps.tile([P, NW], FP, tag="f1", name=f"f1p{hc}{nn}")
                for kc in range(KC):
                    te.matmul(ps, lhsT=(w1[:, kc, :]),
                              rhs=(h2_fm[:, kc, nn * NW:(nn + 1) * NW]),
                              start=kc == 0, stop=kc == KC - 1)
                s.activation(out=g_fm[:, hc, nn * NW:(nn + 1) * NW], in_=ps,
                             func=AF.Gelu, bias=fc1b_t[:, hc:hc + 1], scale=1.0)

            with ExitStack() as sb2:
                tpsum2 = sb2.enter_context(
                    tc.tile_pool(name="tp2", bufs=2, space="PSUM"))
                pps = sb2.enter_context(
                    tc.tile_pool(name="pps", bufs=2, space="PSUM"))
                ptmp_pool = sb2.enter_context(tc.tile_pool(name="ptmp", bufs=3))
                pbp = sb2.enter_context(tc.tile_pool(name="pbp", bufs=1))
                projb_t = bcast_c(pbp, "proj_b") if proj_bias else None
                pscale = 1.0 / (S_V * S_W)
                for tcx in range(TC_N):
                    pss = [pps.tile([P, CS_W], FP, tag=f"pps{cs}",
                                    name=f"pp{tcx}{cs}") for cs in range(2)]
                    for kcp in range(KCP):
                        for cs in range(2):  # consecutive matmuls share lhsT
                            te.matmul(pss[cs],
                                      lhsT=(attn_fm[:, 2 * kcp:2 * kcp + 2,
                                                    tcx * P:(tcx + 1) * P]),
                                      rhs=(pw[:, 2 * kcp:2 * kcp + 2,
                                              cs * CS_W:(cs + 1) * CS_W]),
                                      start=kcp == 0, stop=kcp == KCP - 1,
                                      perf_mode=DR)
                    # fc1 first-half token slice interleaves here: its MMs
                    # fill the PE while the proj->LN2 cross-engine chain for
                    # this chunk runs on Scalar/GpSimd/DVE.
                    if tcx >= 4:
                        for hc in range((tcx - 4) * 6, (tcx - 3) * 6):
                            fc1_chunk(hc, 0, 0)
                    for cs in range(2):
                        ps = pss[cs]
                        xsl = x_tm[:, tcx, cs * CS_W:(cs + 1) * CS_W]
                        ptmp = ptmp_pool.tile([P, CS_W], BF, tag=f"pt{cs}",
                                              name=f"pt{tcx}{cs}")
                        s.activation(out=ptmp, in_=ps, func=AF.Copy,
                                     scale=pscale)
                        if projb_t is not None:
                            v.tensor_add(out=ptmp, in0=ptmp,
                                         in1=projb_t[:, cs * CS_W:(cs + 1) * CS_W])
                        addeng = v if os.environ.get("KNOGPADD") else gp
                        addeng.tensor_add(out=xsl, in0=ptmp, in1=xsl)
                    ln_chunk(2, tcx, x_tm, h2_fm, stat2, h2_pool, tpsum2,
                             ln2_g, ln2_b, BF, 1.0)

            for hc in range(HC_N):
                fc1_chunk(hc, 1, 1)

        # fc2 in groups of 6 hidden chunks, accumulate into x_tm
        GRP = 6
        fc2_r = fc2w_d.rearrange("(hc p) c -> p hc c", p=P)
        out_r = out_d.rearrange("(tc p) c -> p tc c", p=P)
        with ExitStack() as sd:
            f2ps = sd.enter_context(tc.tile_pool(name="f2ps", bufs=3, space="PSUM"))
            for grp in range(HC_N // GRP):
                hcs = list(range(grp * GRP, (grp + 1) * GRP))
                w2t = {}
                for hc in hcs:
                    w2t[hc] = w2_pool.tile([P, C], BF, tag="w2", name=f"w2_{hc}")
                    dma.dma_start(out=w2t[hc], in_=fc2_r[:, hc, :])
                for tcx in range(TC_N):
                    pss = [f2ps.tile([P, CS_W], FP, tag=f"f2ps{cs}",
                                     name=f"f2p{grp}{tcx}{cs}") for cs in range(2)]
                    for i, hc in enumerate(hcs):
                        for cs in range(2):  # consecutive matmuls share lhsT
                            te.matmul(
                                pss[cs], lhsT=(g_fm[:, hc, tcx * P:(tcx + 1) * P]),
                                rhs=(w2t[hc][:, cs * CS_W:(cs + 1) * CS_W]),
                                start=i == 0, stop=i == GRP - 1)
                    for cs in range(2):
                        xsl = x_tm[:, tcx, cs * CS_W:(cs + 1) * CS_W]
                        v.tensor_add(out=xsl, in0=pss[cs], in1=xsl)
                    if grp == HC_N // GRP - 1:
                        dma.dma_start(out=out_r[:, tcx, :], in_=x_tm[:, tcx, :])


_CACHE = {}
last_results = None


def _get_nc(ln_affine, proj_bias):
    key = (ln_affine, proj_bias)
    if key not in _CACHE:
        _CACHE[key] = _build(*key)
    return _CACHE[key]


def kernel(x, qkv_w, proj_w, proj_b, ln1_g, ln1_b, ln2_g, ln2_b,
           fc1_w, fc1_b, fc2_w, fc2_b):
    global last_results
    import ml_dtypes
    from concourse.bass_utils import run_bass_kernel_spmd

    F8NP = ml_dtypes.float8_e4m3
    BFNP = ml_dtypes.bfloat16
    f32 = lambda a: np.ascontiguousarray(np.asarray(a), dtype=np.float32)
    x = f32(x)
    proj_b, fc1_b, fc2_b = map(f32, (proj_b, fc1_b, fc2_b))
    ln1_g, ln1_b, ln2_g, ln2_b = map(f32, (ln1_g, ln1_b, ln2_g, ln2_b))
    qkv_w8 = np.ascontiguousarray(
        (np.asarray(qkv_w, np.float32) * S_W).astype(F8NP))
    proj_w8 = np.ascontiguousarray(
        (np.asarray(proj_w, np.float32) * S_W).astype(F8NP))
    fc1_wb = np.ascontiguousarray(np.asarray(fc1_w, np.float32).astype(BFNP))
    fc2_wb = np.ascontiguousarray(np.asarray(fc2_w, np.float32).astype(BFNP))

    ln_affine = not (np.all(ln1_g == 1) and np.all(ln1_b == 0)
                     and np.all(ln2_g == 1) and np.all(ln2_b == 0))
    proj_bias = bool(np.any(proj_b != 0))
    nc = _get_nc(ln_affine, proj_bias)

    common = {"qkv_w": qkv_w8, "proj_w": proj_w8, "fc1_w": fc1_wb,
              "fc2_w": fc2_wb, "fc1_b": fc1_b}
    if ln_affine:
        common.update({"ln1_g": ln1_g, "ln1_b": ln1_b,
                       "ln2_g": ln2_g, "ln2_b": ln2_b})
    if proj_bias:
        common["proj_b"] = proj_b
    in_maps = [dict(common, x=np.ascontiguousarray(x[b])) for b in range(B)]

    res = run_bass_kernel_spmd(nc, in_maps, core_ids=list(range(B)))
    last_results = res
    out = np.stack([r["out"] for r in res.results], axis=0)
    # fc2_b commutes past the final residual add — fold on host.
    return (out + fc2_b[None, None, :]).astype(np.float32)
